# revision 6
# baseline (speedup 1.0000x reference)
"""Trainium2 Bass kernel v3: u8-quantized output, fp16 K=12 matmuls.

Math per batch row b: root transform Tg from qpos[0:6] (Rodrigues), then per
chain the sequential composition L_j = L_{j-1} @ M_j with
M_j = P0 + sin(q)P1 + cos(q)P2 (host-precomputed P0/P1/P2 from offsets/axes).
Vertex transform out[b,(v,x)] = sum_k A[k,b] W[k,(v,x)] as one K=12 fp16
matmul per (chain,joint), W built on host from verts (x16 quant scale folded
in).  Output is stored u8: psum = 16*out, u8 = round(psum + 128); host
dequantizes (q-128)/16.  Error budget: u8 step 1/16 -> max 0.031 abs vs the
2e-2*absmax = 0.104 gate.

Engine layout per 128-row batch tile (bt):
  ACT    sin/cos (+range-reduction affine ops), ~half the psum->u8 copies
  DVE    reciprocal, L->f16 casts, ~half the psum->u8 copies
  GPSIMD build_M, hom_mul composition, root-transform assembly
  PE     3x N=512 fp16 matmuls per (chain,joint)
  DMA    sync ring: u8 output stream; scalar ring: qpos loads + A transposes
         (dma_start_transpose [128,128] f16, chains at partitions 0/32/64)
Sharding: pure data-parallel over batch, 8 cores x 512 rows.
"""
import math
import numpy as np
from contextlib import ExitStack

import concourse.bass as bass
import concourse.mybir as mybir
import concourse.tile as tile
from concourse import bacc
from concourse.bass_utils import run_bass_kernel_spmd

F32 = mybir.dt.float32
F16 = mybir.dt.float16
U8 = mybir.dt.uint8
AX = mybir.AxisListType
OP = mybir.AluOpType
AF = mybir.ActivationFunctionType

N_CHAINS, N_JOINTS, N_VERTS = 5, 4, 512
NLINK = N_CHAINS * N_JOINTS          # 20
VX = N_VERTS * 3                     # 1536
ROW = NLINK * VX                     # 30720
B_FULL = 4096
N_CORES = 8
B_CORE = B_FULL // N_CORES           # 512
P = 128
NB = B_CORE // P                     # 4 batch tiles per core
TWO_PI = float(np.float32(2.0 * math.pi))
INV_2PI = float(np.float32(1.0 / (2.0 * math.pi)))
HALF_PI = float(np.float32(0.5 * math.pi))
MAGIC = 12582912.0                   # 1.5 * 2**23: fp32 round-to-nearest trick
QS = 16.0                            # output quant scale (folded into W)

MM_MODE = "u8"
REPEAT = 1
LOOP_MODE = "u16"
# copy engine split: copy_i % copy_mod == 0 -> vector, else scalar
COPY_MOD = 2
COPY_COLS = 2048                     # psO tile width (psum->u8 copy width)
DMAS_PER_BT = 2                      # output DMAs per batch tile (sync ring)
TR_ENGS = ("scalar",)                # ring(s) issuing the A transposes
OSTAGE_BUFS = 2


def _view(t, off, dims):
    """Custom free-dim view of a tile AP: keep partition pair, replace free dims."""
    ap = [list(t.ap[0])] + [[s, c] for (s, c) in dims]
    return bass.AP(t.tensor, t.offset + off, ap)


def _host_constants(offsets, axes, verts):
    off = offsets.astype(np.float64)
    ax = axes.astype(np.float64)
    K = np.zeros((N_CHAINS, N_JOINTS, 4, 4))
    x, y, z = ax[..., 0], ax[..., 1], ax[..., 2]
    K[..., 0, 1] = -z; K[..., 0, 2] = y
    K[..., 1, 0] = z;  K[..., 1, 2] = -x
    K[..., 2, 0] = -y; K[..., 2, 1] = x
    K2 = K @ K
    offK = off @ K
    offK2 = off @ K2
    pcon = np.stack([off + offK2, offK, -offK2], 0).reshape(3, NLINK, 16)
    pcon = np.ascontiguousarray(pcon, np.float32)

    W = np.zeros((13, NLINK, VX), np.float32)
    vv = verts.reshape(NLINK, N_VERTS, 3)
    for xx in range(3):
        for l in range(3):
            W[xx * 4 + l, :, xx::3] = vv[:, :, l]
        W[xx * 4 + 3, :, xx::3] = 1.0
    W *= QS
    # 13th row: the u8 bias, delivered via the matmul (A row 12 == 1.0) so
    # psum = QS*out + 128.5 needs no engine-side bias op (the ACT affine
    # stage is low-precision at magnitude ~256 and wrecks rounding).
    W[12] = 128.5
    return pcon, W


def _build_nc(mm_mode, repeat, copy_mod=None, copy_cols=None, dmas_per_bt=None,
              tr_engs=None, ostage_bufs=None, loop_mode=None):
    copy_mod = COPY_MOD if copy_mod is None else copy_mod
    copy_cols = COPY_COLS if copy_cols is None else copy_cols
    dmas_per_bt = DMAS_PER_BT if dmas_per_bt is None else dmas_per_bt
    tr_engs = TR_ENGS if tr_engs is None else tr_engs
    ostage_bufs = OSTAGE_BUFS if ostage_bufs is None else ostage_bufs
    loop_mode = LOOP_MODE if loop_mode is None else loop_mode
    assert mm_mode == "u8"
    assert ROW % copy_cols == 0 and copy_cols % 512 == 0
    n_copy_bt = ROW // copy_cols
    nc = bacc.Bacc("TRN2", target_bir_lowering=False, debug=False)

    qpos = nc.dram_tensor("qpos", [B_CORE, 26], F32, kind="ExternalInput")
    pcon = nc.dram_tensor("pcon", [3 * NLINK * 16], F32, kind="ExternalInput")
    wmat = nc.dram_tensor("wmat", [13, NLINK * VX], F16, kind="ExternalInput")
    out = nc.dram_tensor("out", [B_CORE, ROW], U8, kind="ExternalOutput")

    with tile.TileContext(nc) as tc, ExitStack() as ctx:
        const = ctx.enter_context(tc.tile_pool(name="const", bufs=1))
        qp_pool = ctx.enter_context(tc.tile_pool(name="qp", bufs=2))
        small = ctx.enter_context(tc.tile_pool(name="small", bufs=2))
        tpool = ctx.enter_context(tc.tile_pool(name="tpool", bufs=2))
        mpool = ctx.enter_context(tc.tile_pool(name="mpool", bufs=2))
        lf_pool = ctx.enter_context(tc.tile_pool(name="lf", bufs=4))
        at_pool = ctx.enter_context(tc.tile_pool(name="at", bufs=4))
        ostage = ctx.enter_context(tc.tile_pool(name="ostage", bufs=ostage_bufs))
        psO = ctx.enter_context(tc.tile_pool(name="psO", bufs=2, space="PSUM"))

        # ---- constants ----
        pt = const.tile([P, 3 * NLINK * 16], F32, name="pt")  # broadcast P0/P1/P2
        nc.gpsimd.dma_start(out=pt, in_=bass.AP(pcon, 0, [[0, P], [1, 3 * NLINK * 16]]))

        # W replicated at partition bases 0/32/64: matmul requires
        # lhsT.base_partition() == rhs.base_partition().
        w_sb = const.tile([77, NLINK * VX], F16, name="w_sb")
        for b in (0, 32, 64):
            nc.sync.dma_start(out=w_sb[b:b + 13, :], in_=wmat[:])

        eps_c = const.tile([P, 1], F32, name="eps_c")
        nc.vector.memset(eps_c, 1e-16)
        hpi_c = const.tile([P, 1], F32, name="hpi_c")
        nc.vector.memset(hpi_c, HALF_PI)

        # Prefill the Lf16 transpose-source buffers once so the [P,128]
        # DMA-transposes never read undefined SBUF (only cols holding real
        # chain data are consumed downstream as lhsT).
        # Prefill also plants the constant 1.0 columns (12/44/76) that become
        # the A row driving W's 13th (bias) row after transpose; the per-j
        # casts only touch cols base..base+11 so these survive buffer reuse.
        lf_prefill = []
        for i in range(8):
            t = lf_pool.tile([P, 128], F16, name="lf", tag="lf", bufs=8)
            nc.vector.memset(t, 0.0)
            for col in (12, 44, 76):
                nc.vector.memset(_view(t, col, [(1, 1)]), 1.0)
            lf_prefill.append(t)

        def emit_bt(bt):
            # ---- load qpos tile into cols 0:26; col 26 gets the root angle
            qp = qp_pool.tile([P, 27], F32, name="qp")
            nc.scalar.dma_start(out=qp[:, 0:26], in_=qpos[bt * P:(bt + 1) * P, :])

            # ---- root angle: ang = sqrt(|aa|^2 + tiny) -> qp[:,26] ----
            aasq = small.tile([P, 3], F32, name="aasq")
            s2 = small.tile([P, 1], F32, name="s2")
            nc.scalar.activation(aasq, qp[:, 3:6], AF.Square, accum_out=s2)
            ang = _view(qp, 26, [(1, 1)])
            nc.scalar.activation(ang, s2, AF.Sqrt, bias=eps_c)
            inv = small.tile([P, 1], F32, name="inv")
            nc.vector.reciprocal(inv, ang)
            axs = small.tile([P, 3], F32, name="axs")
            nc.gpsimd.tensor_scalar_mul(axs, qp[:, 3:6], inv)

            # ---- range-reduced sin/cos of [q(20), root_angle] ----
            # r = x - 2pi*round(x/2pi) via the fp32 MAGIC trick; the
            # (t-MAGIC)*2pi step must stay separate for exact cancellation.
            x = qp[:, 6:27]
            sinv = small.tile([P, 21], F32, name="sinv")
            cosv = small.tile([P, 21], F32, name="cosv")
            ts0 = small.tile([P, 21], F32, name="ts0")
            ts1 = small.tile([P, 21], F32, name="ts1")
            nc.scalar.activation(ts0, x, AF.Copy, bias=MAGIC, scale=INV_2PI)
            nc.gpsimd.tensor_scalar(ts0, ts0, MAGIC, TWO_PI, OP.subtract, OP.mult)
            nc.gpsimd.tensor_sub(ts0, x, ts0)
            nc.scalar.activation(sinv, ts0, AF.Sin)
            # cos(x) = sin(pi/2 - x); a fused bias of MAGIC+0.25 would lose
            # the 0.25 to fp32 rounding (ulp(MAGIC)=1), so reduce xp = pi/2-x
            # through its own exact-cancellation path.
            xp = small.tile([P, 21], F32, name="xp")
            nc.scalar.activation(xp, x, AF.Copy, bias=HALF_PI, scale=-1.0)
            nc.scalar.activation(ts1, xp, AF.Copy, bias=MAGIC, scale=INV_2PI)
            nc.gpsimd.tensor_scalar(ts1, ts1, MAGIC, TWO_PI, OP.subtract, OP.mult)
            nc.gpsimd.tensor_sub(ts1, xp, ts1)
            nc.scalar.activation(cosv, ts1, AF.Sin)

            s_r = _view(sinv, 20, [(1, 1)])
            c_r = _view(cosv, 20, [(1, 1)])

            # ---- M_j for ALL joints: [P, 240] link-major (c*4+j)*12 ----
            M = mpool.tile([P, 240], F32, name="M", tag="M", bufs=2)
            Mt = mpool.tile([P, 240], F32, name="Mt", tag="Mt", bufs=2)
            Mv = _view(M, 0, [(12, 20), (1, 12)])
            Mtv = _view(Mt, 0, [(12, 20), (1, 12)])
            P0v = _view(pt, 0, [(16, 20), (1, 12)])
            P1v = _view(pt, 320, [(16, 20), (1, 12)])
            P2v = _view(pt, 640, [(16, 20), (1, 12)])
            sv = _view(sinv, 0, [(1, 20), (0, 12)])
            cv = _view(cosv, 0, [(1, 20), (0, 12)])
            nc.gpsimd.tensor_mul(Mv, P1v, sv)
            nc.gpsimd.tensor_mul(Mtv, P2v, cv)
            nc.gpsimd.tensor_add(Mv, Mv, P0v)
            nc.gpsimd.tensor_add(Mv, Mv, Mtv)

            # ---- root transform Tg [P, 12] (cols x*4+m) ----
            omc = small.tile([P, 1], F32, name="omc")
            nc.gpsimd.tensor_scalar(omc, c_r, -1.0, 1.0, OP.mult, OP.add)
            outer = small.tile([P, 9], F32, name="outer")
            nc.gpsimd.tensor_mul(
                _view(outer, 0, [(3, 3), (1, 3)]),
                _view(axs, 0, [(1, 3), (0, 3)]),
                _view(axs, 0, [(0, 3), (1, 3)]),
            )
            Tg = small.tile([P, 12], F32, name="Tg")
            nc.gpsimd.tensor_scalar_mul(
                _view(Tg, 0, [(4, 3), (1, 3)]),
                _view(outer, 0, [(3, 3), (1, 3)]),
                omc,
            )
            nc.gpsimd.tensor_scalar_add(
                _view(Tg, 0, [(5, 3)]), _view(Tg, 0, [(5, 3)]), c_r
            )
            sa = small.tile([P, 3], F32, name="sa")
            nc.gpsimd.tensor_scalar_mul(sa, axs, s_r)
            for (col, k, op) in ((1, 2, OP.subtract), (2, 1, OP.add),
                                 (4, 2, OP.add), (6, 0, OP.subtract),
                                 (8, 1, OP.subtract), (9, 0, OP.add)):
                v = _view(Tg, col, [(1, 1)])
                nc.gpsimd.tensor_tensor(v, v, _view(sa, k, [(1, 1)]), op)
            nc.gpsimd.tensor_copy(_view(Tg, 3, [(4, 3)]), qp[:, 0:3])

            # ---- homogeneous product -> L-format [P,60] (12 cols/chain) ----
            def hom_mul(A, Bj, A_fmt):
                Tn = tpool.tile([P, 60], F32, name="L", tag="L", bufs=8)
                Tnv = _view(Tn, 0, [(12, 5), (4, 3), (1, 4)])
                Tt = tpool.tile([P, 60], F32, name="Ltmp", tag="Ltmp", bufs=2)
                Ttv = _view(Tt, 0, [(12, 5), (4, 3), (1, 4)])

                def a_view(m):
                    if A_fmt == "G":
                        return _view(A, m, [(0, 5), (4, 3), (0, 4)])
                    return _view(A, m, [(12, 5), (4, 3), (0, 4)])

                def b_view(m):
                    # M slice for joint j: chain stride 48, entry (j*12 + m*4)
                    return _view(M, Bj * 12 + m * 4, [(48, 5), (0, 3), (1, 4)])

                nc.gpsimd.tensor_mul(Tnv, a_view(0), b_view(0))
                nc.gpsimd.tensor_mul(Ttv, a_view(1), b_view(1))
                nc.gpsimd.tensor_add(Tnv, Tnv, Ttv)
                nc.gpsimd.tensor_mul(Ttv, a_view(2), b_view(2))
                nc.gpsimd.tensor_add(Tnv, Tnv, Ttv)
                t3o = _view(Tn, 3, [(12, 5), (4, 3)])
                if A_fmt == "G":
                    t3i = _view(A, 3, [(0, 5), (4, 3)])
                else:
                    t3i = _view(A, 3, [(12, 5), (4, 3)])
                nc.gpsimd.tensor_tensor(t3o, t3o, t3i, OP.add)
                return Tn

            # ---- cast to f16 transpose sources + DMA-transpose to lhsT ----
            # Lf16a holds chains 0-2 at cols 0/32/64, Lf16b chains 3-4 at
            # 0/32 (PE lhsT base partition must be 0/32/64).
            def stage_transpose(L):
                lfa = lf_pool.tile([P, 128], F16, name="lf", tag="lf", bufs=8)
                lfb = lf_pool.tile([P, 128], F16, name="lf", tag="lf", bufs=8)
                nc.vector.tensor_copy(
                    _view(lfa, 0, [(32, 3), (1, 12)]),
                    _view(L, 0, [(12, 3), (1, 12)]))
                nc.vector.tensor_copy(
                    _view(lfb, 0, [(32, 2), (1, 12)]),
                    _view(L, 36, [(12, 2), (1, 12)]))
                ata = at_pool.tile([P, 128], F16, name="ata", tag="at", bufs=12)
                atb = at_pool.tile([P, 128], F16, name="atb", tag="at", bufs=12)
                eng = getattr(nc, tr_engs[tr_state[0] % len(tr_engs)])
                tr_state[0] += 1
                eng.dma_start(out=ata, in_=lfa, transpose=True)
                eng2 = getattr(nc, tr_engs[tr_state[0] % len(tr_engs)])
                tr_state[0] += 1
                eng2.dma_start(out=atb, in_=lfb, transpose=True)
                return ata, atb

            # ---- all transforms + transposes first (j-sequential), then
            # matmuls chain-outer/joint-inner so psum chunk index g runs
            # consecutively through the link-major ostage layout.
            ost = ostage.tile([P, ROW], U8, name="ost")
            ats = []
            L = hom_mul(Tg, 0, "G")
            ats.append(stage_transpose(L))
            for j in range(1, N_JOINTS):
                L = hom_mul(L, j, "T")
                ats.append(stage_transpose(L))

            cpt = copy_cols // 512
            for c in range(N_CHAINS):
                at_sel = c if c < 3 else c - 3
                base = 32 * at_sel
                for j in range(N_JOINTS):
                    ata, atb = ats[j]
                    at = ata if c < 3 else atb
                    lhsT = at[base:base + 13, :]
                    link = c * N_JOINTS + j
                    for i in range(3):
                        g = link * 3 + i
                        ti, slot = divmod(g, cpt)
                        if slot == 0:
                            ps_state[ti % 2] = psO.tile(
                                [P, copy_cols], F32, name="O_ps", space="PSUM")
                        ps = ps_state[ti % 2]
                        wv = _view(w_sb[base:base + 13, :],
                                   link * VX + i * 512, [(1, 512)])
                        nc.tensor.matmul(ps[:, slot * 512:(slot + 1) * 512],
                                         lhsT, wv)
                        if slot == cpt - 1:
                            # psum is pre-biased (+128.5 via W row 12), so
                            # both paths are plain casts: DVE truncates,
                            # ACT rounds-to-nearest (host undoes per tile).
                            oslc = ost[:, ti * copy_cols:(ti + 1) * copy_cols]
                            if copy_state[0] % copy_mod == 0:
                                nc.vector.tensor_copy(oslc, ps)
                            else:
                                nc.scalar.copy(oslc, ps)
                            copy_state[0] += 1

            # ---- output stream: contiguous u8 rows on the sync ring ----
            cw = ROW // dmas_per_bt
            for d in range(dmas_per_bt):
                dst = bass.AP(out, (bt * P) * ROW + d * cw, [[ROW, P], [1, cw]])
                nc.sync.dma_start(out=dst, in_=ost[:, d * cw:(d + 1) * cw])

        copy_state = [0]
        tr_state = [0]
        ps_state = [None, None]

        if repeat == 1:
            for bt in range(NB):
                emit_bt(bt)
        elif loop_mode.startswith("u"):
            U = int(loop_mode[1:])
            M_ = (repeat - 1) // U
            assert M_ * U + 1 == repeat, (repeat, U)
            with tc.For_i(0, M_, 1):
                for _ in range(U):
                    for bt in range(NB):
                        emit_bt(bt)
            for bt in range(NB):
                emit_bt(bt)
        else:
            with tc.For_i(0, repeat, 1):
                for bt in range(NB):
                    emit_bt(bt)

    nc.compile()
    return nc


_NC_CACHE = {}


def _get_nc(mm_mode=None, repeat=None, copy_mod=None, copy_cols=None,
            dmas_per_bt=None, tr_engs=None, ostage_bufs=None, loop_mode=None):
    mm_mode = MM_MODE if mm_mode is None else mm_mode
    repeat = REPEAT if repeat is None else repeat
    copy_mod = COPY_MOD if copy_mod is None else copy_mod
    copy_cols = COPY_COLS if copy_cols is None else copy_cols
    dmas_per_bt = DMAS_PER_BT if dmas_per_bt is None else dmas_per_bt
    tr_engs = TR_ENGS if tr_engs is None else tr_engs
    ostage_bufs = OSTAGE_BUFS if ostage_bufs is None else ostage_bufs
    loop_mode = LOOP_MODE if loop_mode is None else loop_mode
    key = (mm_mode, repeat, copy_mod, copy_cols, dmas_per_bt, tuple(tr_engs),
           ostage_bufs, loop_mode)
    if key not in _NC_CACHE:
        _NC_CACHE[key] = _build_nc(mm_mode, repeat, copy_mod, copy_cols,
                                   dmas_per_bt, tr_engs, ostage_bufs, loop_mode)
    return _NC_CACHE[key]


def _make_in_maps(qpos, offsets, axes, verts, mm_mode="u8"):
    import ml_dtypes
    qpos = np.ascontiguousarray(qpos, np.float32)
    pcon, W = _host_constants(np.asarray(offsets, np.float32),
                              np.asarray(axes, np.float32),
                              np.asarray(verts, np.float32))
    pcon_flat = np.ascontiguousarray(pcon.reshape(-1))
    Wm = np.ascontiguousarray(
        W.reshape(13, NLINK * VX).astype(np.float16))
    return [
        {"qpos": np.ascontiguousarray(qpos[i * B_CORE:(i + 1) * B_CORE]),
         "pcon": pcon_flat, "wmat": Wm}
        for i in range(N_CORES)
    ]


def kernel(qpos, offsets, axes, verts):
    nc = _get_nc()
    in_maps = _make_in_maps(qpos, offsets, axes, verts, MM_MODE)
    res = run_bass_kernel_spmd(nc, in_maps, core_ids=list(range(N_CORES)))
    outs = [res.results[i]["out"] for i in range(N_CORES)]
    # psum = QS*out + 128.5; both engines' plain f32->u8 casts round to
    # nearest (measured), so q = rne(psum) and reconstruction is q - 128.5.
    full = np.concatenate(outs, axis=0).astype(np.float32)
    full -= 128.5
    full *= 1.0 / QS
    return full.reshape(B_FULL, N_CHAINS, N_JOINTS, N_VERTS, 3)


# revision 17
# speedup vs baseline: 1.3335x; 1.3335x over previous
"""Trainium2 Bass kernel v3: u8-quantized output, fp16 K=12 matmuls.

Math per batch row b: root transform Tg from qpos[0:6] (Rodrigues), then per
chain the sequential composition L_j = L_{j-1} @ M_j with
M_j = P0 + sin(q)P1 + cos(q)P2 (host-precomputed P0/P1/P2 from offsets/axes).
Vertex transform out[b,(v,x)] = sum_k A[k,b] W[k,(v,x)] as one K=12 fp16
matmul per (chain,joint), W built on host from verts (x16 quant scale folded
in).  Output is stored u8: psum = 16*out, u8 = round(psum + 128); host
dequantizes (q-128)/16.  Error budget: u8 step 1/16 -> max 0.031 abs vs the
2e-2*absmax = 0.104 gate.

Engine layout per 128-row batch tile (bt):
  ACT    sin/cos (+range-reduction affine ops), ~half the psum->u8 copies
  DVE    reciprocal, L->f16 casts, ~half the psum->u8 copies
  GPSIMD build_M, hom_mul composition, root-transform assembly
  PE     3x N=512 fp16 matmuls per (chain,joint)
  DMA    sync ring: u8 output stream; scalar ring: qpos loads + A transposes
         (dma_start_transpose [128,128] f16, chains at partitions 0/32/64)
Sharding: pure data-parallel over batch, 8 cores x 512 rows.
"""
import math
import numpy as np
from contextlib import ExitStack

import concourse.bass as bass
import concourse.mybir as mybir
import concourse.tile as tile
from concourse import bacc
from concourse.bass_utils import run_bass_kernel_spmd
from concourse.masks import make_identity

F32 = mybir.dt.float32
F16 = mybir.dt.float16
U8 = mybir.dt.uint8
AX = mybir.AxisListType
OP = mybir.AluOpType
AF = mybir.ActivationFunctionType

N_CHAINS, N_JOINTS, N_VERTS = 5, 4, 512
NLINK = N_CHAINS * N_JOINTS          # 20
VX = N_VERTS * 3                     # 1536
ROW = NLINK * VX                     # 30720
B_FULL = 4096
N_CORES = 8
B_CORE = B_FULL // N_CORES           # 512
P = 128
NB = B_CORE // P                     # 4 batch tiles per core
TWO_PI = float(np.float32(2.0 * math.pi))
INV_2PI = float(np.float32(1.0 / (2.0 * math.pi)))
HALF_PI = float(np.float32(0.5 * math.pi))
MAGIC = 12582912.0                   # 1.5 * 2**23: fp32 round-to-nearest trick
QS = 16.0                            # output quant scale (folded into W)

MM_MODE = "u8"
REPEAT = 1
LOOP_MODE = "u16"
# copy engine split: copy_i % copy_mod == 0 -> vector, else scalar
COPY_MOD = 2
COPY_COLS = 1024                     # psO tile width (psum->u8 copy width)
DMAS_PER_BT = 2                      # output DMAs per batch tile (sync ring)
TR_ENGS = ("scalar",)                # ring(s) issuing the A transposes
TR_MODE = "pe"                       # "dma" (xbar) or "pe" (PE transpose)
CAST_ENG = "gpsimd"                  # engine for the L->f16 casts
QP_ENG = "gpsimd"                    # ring for the qpos loads
OSTAGE_BUFS = 2


def _view(t, off, dims):
    """Custom free-dim view of a tile AP: keep partition pair, replace free dims."""
    ap = [list(t.ap[0])] + [[s, c] for (s, c) in dims]
    return bass.AP(t.tensor, t.offset + off, ap)


def _host_constants(offsets, axes, verts):
    off = offsets.astype(np.float64)
    ax = axes.astype(np.float64)
    K = np.zeros((N_CHAINS, N_JOINTS, 4, 4))
    x, y, z = ax[..., 0], ax[..., 1], ax[..., 2]
    K[..., 0, 1] = -z; K[..., 0, 2] = y
    K[..., 1, 0] = z;  K[..., 1, 2] = -x
    K[..., 2, 0] = -y; K[..., 2, 1] = x
    K2 = K @ K
    offK = off @ K
    offK2 = off @ K2
    pcon = np.stack([off + offK2, offK, -offK2], 0).reshape(3, NLINK, 16)
    pcon = np.ascontiguousarray(pcon, np.float32)

    W = np.zeros((13, NLINK, VX), np.float32)
    vv = verts.reshape(NLINK, N_VERTS, 3)
    for xx in range(3):
        for l in range(3):
            W[xx * 4 + l, :, xx::3] = vv[:, :, l]
        W[xx * 4 + 3, :, xx::3] = 1.0
    W *= QS
    # 13th row: the u8 bias, delivered via the matmul (A row 12 == 1.0) so
    # psum = QS*out + 128.5 needs no engine-side bias op (the ACT affine
    # stage is low-precision at magnitude ~256 and wrecks rounding).
    W[12] = 128.5
    return pcon, W


def _build_nc(mm_mode, repeat, copy_mod=None, copy_cols=None, dmas_per_bt=None,
              tr_engs=None, ostage_bufs=None, loop_mode=None, tr_mode=None,
              cast_eng=None, qp_eng=None):
    copy_mod = COPY_MOD if copy_mod is None else copy_mod
    copy_cols = COPY_COLS if copy_cols is None else copy_cols
    dmas_per_bt = DMAS_PER_BT if dmas_per_bt is None else dmas_per_bt
    tr_engs = TR_ENGS if tr_engs is None else tr_engs
    ostage_bufs = OSTAGE_BUFS if ostage_bufs is None else ostage_bufs
    loop_mode = LOOP_MODE if loop_mode is None else loop_mode
    tr_mode = TR_MODE if tr_mode is None else tr_mode
    cast_eng = CAST_ENG if cast_eng is None else cast_eng
    qp_eng = QP_ENG if qp_eng is None else qp_eng
    if tr_mode == "pe":
        assert copy_cols in (512, 1024, 1536)
    assert mm_mode == "u8"
    assert ROW % copy_cols == 0 and copy_cols % 512 == 0
    n_copy_bt = ROW // copy_cols
    nc = bacc.Bacc("TRN2", target_bir_lowering=False, debug=False)

    qpos = nc.dram_tensor("qpos", [B_CORE, 26], F32, kind="ExternalInput")
    pcon = nc.dram_tensor("pcon", [3 * NLINK * 16], F32, kind="ExternalInput")
    wmat = nc.dram_tensor("wmat", [13, NLINK * VX], F16, kind="ExternalInput")
    out = nc.dram_tensor("out", [B_CORE, ROW], U8, kind="ExternalOutput")

    with tile.TileContext(nc) as tc, ExitStack() as ctx:
        const = ctx.enter_context(tc.tile_pool(name="const", bufs=1))
        qp_pool = ctx.enter_context(tc.tile_pool(name="qp", bufs=2))
        small = ctx.enter_context(tc.tile_pool(name="small", bufs=2))
        tpool = ctx.enter_context(tc.tile_pool(name="tpool", bufs=2))
        mpool = ctx.enter_context(tc.tile_pool(name="mpool", bufs=2))
        lf_pool = ctx.enter_context(tc.tile_pool(name="lf", bufs=4))
        at_pool = ctx.enter_context(tc.tile_pool(name="at", bufs=4))
        ostage = ctx.enter_context(tc.tile_pool(name="ostage", bufs=ostage_bufs))
        n_bank = (copy_cols * 4) // 2048
        avail = 8 - (2 if tr_mode == "pe" else 0)
        pso_bufs = avail // n_bank
        psO = ctx.enter_context(tc.tile_pool(name="psO", bufs=pso_bufs,
                                             space="PSUM"))
        if tr_mode == "pe":
            psA = ctx.enter_context(tc.tile_pool(name="psA", bufs=2,
                                                 space="PSUM"))

        # ---- constants ----
        pt = const.tile([P, 3 * NLINK * 16], F32, name="pt")  # broadcast P0/P1/P2
        nc.gpsimd.dma_start(out=pt, in_=bass.AP(pcon, 0, [[0, P], [1, 3 * NLINK * 16]]))

        # W replicated at partition bases 0/32/64: matmul requires
        # lhsT.base_partition() == rhs.base_partition().
        w_sb = const.tile([77, NLINK * VX], F16, name="w_sb")
        for b in (0, 32, 64):
            nc.sync.dma_start(out=w_sb[b:b + 13, :], in_=wmat[:])

        eps_c = const.tile([P, 1], F32, name="eps_c")
        nc.vector.memset(eps_c, 1e-16)
        hpi_c = const.tile([P, 1], F32, name="hpi_c")
        nc.vector.memset(hpi_c, HALF_PI)
        if tr_mode == "pe":
            ident16 = const.tile([P, P], F16, name="ident16")
            make_identity(nc, ident16)

        # Prefill the Lf16 transpose-source buffers once so the [P,128]
        # DMA-transposes never read undefined SBUF (only cols holding real
        # chain data are consumed downstream as lhsT).
        # Prefill also plants the constant 1.0 columns (12/44/76) that become
        # the A row driving W's 13th (bias) row after transpose; the per-j
        # casts only touch cols base..base+11 so these survive buffer reuse.
        lf_cols = 128 if tr_mode == "dma" else 65
        ones_cols = (12, 44, 76) if tr_mode == "dma" else (12, 25, 38, 51, 64)
        lf_prefill = []
        for i in range(8):
            t = lf_pool.tile([P, lf_cols], F16, name="lf", tag="lf", bufs=8)
            nc.vector.memset(t, 0.0)
            for col in ones_cols:
                nc.vector.memset(_view(t, col, [(1, 1)]), 1.0)
            lf_prefill.append(t)

        def emit_bt(bt):
            # ---- load qpos tile into cols 0:26; col 26 gets the root angle
            qp = qp_pool.tile([P, 27], F32, name="qp")
            getattr(nc, qp_eng).dma_start(out=qp[:, 0:26],
                                          in_=qpos[bt * P:(bt + 1) * P, :])

            # ---- root angle: ang = sqrt(|aa|^2 + tiny) -> qp[:,26] ----
            aasq = small.tile([P, 3], F32, name="aasq")
            s2 = small.tile([P, 1], F32, name="s2")
            nc.scalar.activation(aasq, qp[:, 3:6], AF.Square, accum_out=s2)
            ang = _view(qp, 26, [(1, 1)])
            nc.scalar.activation(ang, s2, AF.Sqrt, bias=eps_c)
            inv = small.tile([P, 1], F32, name="inv")
            nc.vector.reciprocal(inv, ang)
            axs = small.tile([P, 3], F32, name="axs")
            nc.gpsimd.tensor_scalar_mul(axs, qp[:, 3:6], inv)

            # ---- range-reduced sin/cos of [q(20), root_angle] ----
            # r = x - 2pi*round(x/2pi) via the fp32 MAGIC trick; the
            # (t-MAGIC)*2pi step must stay separate for exact cancellation.
            x = qp[:, 6:27]
            sinv = small.tile([P, 21], F32, name="sinv")
            cosv = small.tile([P, 21], F32, name="cosv")
            ts0 = small.tile([P, 21], F32, name="ts0")
            ts1 = small.tile([P, 21], F32, name="ts1")
            nc.scalar.activation(ts0, x, AF.Copy, bias=MAGIC, scale=INV_2PI)
            nc.gpsimd.tensor_scalar(ts0, ts0, MAGIC, TWO_PI, OP.subtract, OP.mult)
            nc.gpsimd.tensor_sub(ts0, x, ts0)
            nc.scalar.activation(sinv, ts0, AF.Sin)
            # cos(x) = sin(pi/2 - x); a fused bias of MAGIC+0.25 would lose
            # the 0.25 to fp32 rounding (ulp(MAGIC)=1), so reduce xp = pi/2-x
            # through its own exact-cancellation path.
            xp = small.tile([P, 21], F32, name="xp")
            nc.scalar.activation(xp, x, AF.Copy, bias=HALF_PI, scale=-1.0)
            nc.scalar.activation(ts1, xp, AF.Copy, bias=MAGIC, scale=INV_2PI)
            nc.gpsimd.tensor_scalar(ts1, ts1, MAGIC, TWO_PI, OP.subtract, OP.mult)
            nc.gpsimd.tensor_sub(ts1, xp, ts1)
            nc.scalar.activation(cosv, ts1, AF.Sin)

            s_r = _view(sinv, 20, [(1, 1)])
            c_r = _view(cosv, 20, [(1, 1)])

            # ---- M_j for ALL joints: [P, 240] link-major (c*4+j)*12 ----
            M = mpool.tile([P, 240], F32, name="M", tag="M", bufs=2)
            Mt = mpool.tile([P, 240], F32, name="Mt", tag="Mt", bufs=2)
            Mv = _view(M, 0, [(12, 20), (1, 12)])
            Mtv = _view(Mt, 0, [(12, 20), (1, 12)])
            P0v = _view(pt, 0, [(16, 20), (1, 12)])
            P1v = _view(pt, 320, [(16, 20), (1, 12)])
            P2v = _view(pt, 640, [(16, 20), (1, 12)])
            sv = _view(sinv, 0, [(1, 20), (0, 12)])
            cv = _view(cosv, 0, [(1, 20), (0, 12)])
            nc.gpsimd.tensor_mul(Mv, P1v, sv)
            nc.gpsimd.tensor_mul(Mtv, P2v, cv)
            nc.gpsimd.tensor_add(Mv, Mv, P0v)
            nc.gpsimd.tensor_add(Mv, Mv, Mtv)

            # ---- root transform Tg [P, 12] (cols x*4+m) ----
            omc = small.tile([P, 1], F32, name="omc")
            nc.gpsimd.tensor_scalar(omc, c_r, -1.0, 1.0, OP.mult, OP.add)
            outer = small.tile([P, 9], F32, name="outer")
            nc.gpsimd.tensor_mul(
                _view(outer, 0, [(3, 3), (1, 3)]),
                _view(axs, 0, [(1, 3), (0, 3)]),
                _view(axs, 0, [(0, 3), (1, 3)]),
            )
            Tg = small.tile([P, 12], F32, name="Tg")
            nc.gpsimd.tensor_scalar_mul(
                _view(Tg, 0, [(4, 3), (1, 3)]),
                _view(outer, 0, [(3, 3), (1, 3)]),
                omc,
            )
            nc.gpsimd.tensor_scalar_add(
                _view(Tg, 0, [(5, 3)]), _view(Tg, 0, [(5, 3)]), c_r
            )
            sa = small.tile([P, 3], F32, name="sa")
            nc.gpsimd.tensor_scalar_mul(sa, axs, s_r)
            for (col, k, op) in ((1, 2, OP.subtract), (2, 1, OP.add),
                                 (4, 2, OP.add), (6, 0, OP.subtract),
                                 (8, 1, OP.subtract), (9, 0, OP.add)):
                v = _view(Tg, col, [(1, 1)])
                nc.gpsimd.tensor_tensor(v, v, _view(sa, k, [(1, 1)]), op)
            nc.gpsimd.tensor_copy(_view(Tg, 3, [(4, 3)]), qp[:, 0:3])

            # ---- homogeneous product -> L-format [P,60] (12 cols/chain) ----
            def hom_mul(A, Bj, A_fmt):
                Tn = tpool.tile([P, 60], F32, name="L", tag="L", bufs=8)
                Tnv = _view(Tn, 0, [(12, 5), (4, 3), (1, 4)])
                Tt = tpool.tile([P, 60], F32, name="Ltmp", tag="Ltmp", bufs=2)
                Ttv = _view(Tt, 0, [(12, 5), (4, 3), (1, 4)])

                def a_view(m):
                    if A_fmt == "G":
                        return _view(A, m, [(0, 5), (4, 3), (0, 4)])
                    return _view(A, m, [(12, 5), (4, 3), (0, 4)])

                def b_view(m):
                    # M slice for joint j: chain stride 48, entry (j*12 + m*4)
                    return _view(M, Bj * 12 + m * 4, [(48, 5), (0, 3), (1, 4)])

                nc.gpsimd.tensor_mul(Tnv, a_view(0), b_view(0))
                nc.gpsimd.tensor_mul(Ttv, a_view(1), b_view(1))
                nc.gpsimd.tensor_add(Tnv, Tnv, Ttv)
                nc.gpsimd.tensor_mul(Ttv, a_view(2), b_view(2))
                nc.gpsimd.tensor_add(Tnv, Tnv, Ttv)
                t3o = _view(Tn, 3, [(12, 5), (4, 3)])
                if A_fmt == "G":
                    t3i = _view(A, 3, [(0, 5), (4, 3)])
                else:
                    t3i = _view(A, 3, [(12, 5), (4, 3)])
                nc.gpsimd.tensor_tensor(t3o, t3o, t3i, OP.add)
                return Tn

            # ---- cast to f16 transpose sources + DMA-transpose to lhsT ----
            # Lf16a holds chains 0-2 at cols 0/32/64, Lf16b chains 3-4 at
            # 0/32 (PE lhsT base partition must be 0/32/64).
            def stage_transpose(L):
                cast = getattr(nc, cast_eng)
                if tr_mode == "pe":
                    lf = lf_pool.tile([P, 65], F16, name="lf", tag="lf", bufs=8)
                    cast.tensor_copy(
                        _view(lf, 0, [(13, 5), (1, 12)]),
                        _view(L, 0, [(12, 5), (1, 12)]))
                    at_ps = psA.tile([13, 640], F16, name="at_ps", space="PSUM")
                    for c in range(N_CHAINS):
                        nc.tensor.transpose(
                            at_ps[:, c * 128:(c + 1) * 128],
                            _view(lf, c * 13, [(1, 13)]), ident16)
                    at_sb = at_pool.tile([13, 640], F16, name="at_sb",
                                         tag="at", bufs=6)
                    if tr_state[0] % 2 == 0:
                        nc.vector.tensor_copy(at_sb, at_ps)
                    else:
                        nc.scalar.copy(at_sb, at_ps)
                    tr_state[0] += 1
                    return at_sb, at_sb
                lfa = lf_pool.tile([P, 128], F16, name="lf", tag="lf", bufs=8)
                lfb = lf_pool.tile([P, 128], F16, name="lf", tag="lf", bufs=8)
                cast.tensor_copy(
                    _view(lfa, 0, [(32, 3), (1, 12)]),
                    _view(L, 0, [(12, 3), (1, 12)]))
                cast.tensor_copy(
                    _view(lfb, 0, [(32, 2), (1, 12)]),
                    _view(L, 36, [(12, 2), (1, 12)]))
                ata = at_pool.tile([P, 128], F16, name="ata", tag="at", bufs=12)
                atb = at_pool.tile([P, 128], F16, name="atb", tag="at", bufs=12)
                eng = getattr(nc, tr_engs[tr_state[0] % len(tr_engs)])
                tr_state[0] += 1
                eng.dma_start(out=ata, in_=lfa, transpose=True)
                eng2 = getattr(nc, tr_engs[tr_state[0] % len(tr_engs)])
                tr_state[0] += 1
                eng2.dma_start(out=atb, in_=lfb, transpose=True)
                return ata, atb

            # ---- all transforms + transposes first (j-sequential), then
            # matmuls chain-outer/joint-inner so psum chunk index g runs
            # consecutively through the link-major ostage layout.
            ost = ostage.tile([P, ROW], U8, name="ost")
            ats = []
            L = hom_mul(Tg, 0, "G")
            ats.append(stage_transpose(L))
            for j in range(1, N_JOINTS):
                L = hom_mul(L, j, "T")
                ats.append(stage_transpose(L))

            cpt = copy_cols // 512
            for c in range(N_CHAINS):
                at_sel = c if c < 3 else c - 3
                base = 0 if tr_mode == "pe" else 32 * at_sel
                for j in range(N_JOINTS):
                    ata, atb = ats[j]
                    at = ata if c < 3 else atb
                    if tr_mode == "pe":
                        lhsT = at[:, c * 128:(c + 1) * 128]
                    else:
                        lhsT = at[base:base + 13, :]
                    link = c * N_JOINTS + j
                    for i in range(3):
                        g = link * 3 + i
                        ti, slot = divmod(g, cpt)
                        if slot == 0:
                            ps_state[ti % pso_bufs] = psO.tile(
                                [P, copy_cols], F32, name="O_ps", space="PSUM")
                        ps = ps_state[ti % pso_bufs]
                        wv = _view(w_sb[base:base + 13, :],
                                   link * VX + i * 512, [(1, 512)])
                        nc.tensor.matmul(ps[:, slot * 512:(slot + 1) * 512],
                                         lhsT, wv)
                        if slot == cpt - 1:
                            # psum is pre-biased (+128.5 via W row 12), so
                            # both paths are plain casts: DVE truncates,
                            # ACT rounds-to-nearest (host undoes per tile).
                            oslc = ost[:, ti * copy_cols:(ti + 1) * copy_cols]
                            if copy_state[0] % copy_mod == 0:
                                nc.vector.tensor_copy(oslc, ps)
                            else:
                                nc.scalar.copy(oslc, ps)
                            copy_state[0] += 1

            # ---- output stream: contiguous u8 rows on the sync ring ----
            cw = ROW // dmas_per_bt
            for d in range(dmas_per_bt):
                dst = bass.AP(out, (bt * P) * ROW + d * cw, [[ROW, P], [1, cw]])
                nc.sync.dma_start(out=dst, in_=ost[:, d * cw:(d + 1) * cw])

        copy_state = [0]
        tr_state = [0]
        ps_state = [None] * 8

        if repeat == 1:
            for bt in range(NB):
                emit_bt(bt)
        elif loop_mode.startswith("u"):
            U = int(loop_mode[1:])
            M_ = (repeat - 1) // U
            assert M_ * U + 1 == repeat, (repeat, U)
            with tc.For_i(0, M_, 1):
                for _ in range(U):
                    for bt in range(NB):
                        emit_bt(bt)
            for bt in range(NB):
                emit_bt(bt)
        else:
            with tc.For_i(0, repeat, 1):
                for bt in range(NB):
                    emit_bt(bt)

    nc.compile()
    return nc


_NC_CACHE = {}


def _get_nc(mm_mode=None, repeat=None, copy_mod=None, copy_cols=None,
            dmas_per_bt=None, tr_engs=None, ostage_bufs=None, loop_mode=None,
            tr_mode=None, cast_eng=None, qp_eng=None):
    mm_mode = MM_MODE if mm_mode is None else mm_mode
    repeat = REPEAT if repeat is None else repeat
    copy_mod = COPY_MOD if copy_mod is None else copy_mod
    copy_cols = COPY_COLS if copy_cols is None else copy_cols
    dmas_per_bt = DMAS_PER_BT if dmas_per_bt is None else dmas_per_bt
    tr_engs = TR_ENGS if tr_engs is None else tr_engs
    ostage_bufs = OSTAGE_BUFS if ostage_bufs is None else ostage_bufs
    loop_mode = LOOP_MODE if loop_mode is None else loop_mode
    tr_mode = TR_MODE if tr_mode is None else tr_mode
    cast_eng = CAST_ENG if cast_eng is None else cast_eng
    qp_eng = QP_ENG if qp_eng is None else qp_eng
    key = (mm_mode, repeat, copy_mod, copy_cols, dmas_per_bt, tuple(tr_engs),
           ostage_bufs, loop_mode, tr_mode, cast_eng, qp_eng)
    if key not in _NC_CACHE:
        _NC_CACHE[key] = _build_nc(mm_mode, repeat, copy_mod, copy_cols,
                                   dmas_per_bt, tr_engs, ostage_bufs,
                                   loop_mode, tr_mode, cast_eng, qp_eng)
    return _NC_CACHE[key]


def _make_in_maps(qpos, offsets, axes, verts, mm_mode="u8"):
    import ml_dtypes
    qpos = np.ascontiguousarray(qpos, np.float32)
    pcon, W = _host_constants(np.asarray(offsets, np.float32),
                              np.asarray(axes, np.float32),
                              np.asarray(verts, np.float32))
    pcon_flat = np.ascontiguousarray(pcon.reshape(-1))
    Wm = np.ascontiguousarray(
        W.reshape(13, NLINK * VX).astype(np.float16))
    return [
        {"qpos": np.ascontiguousarray(qpos[i * B_CORE:(i + 1) * B_CORE]),
         "pcon": pcon_flat, "wmat": Wm}
        for i in range(N_CORES)
    ]


def kernel(qpos, offsets, axes, verts):
    nc = _get_nc()
    in_maps = _make_in_maps(qpos, offsets, axes, verts, MM_MODE)
    res = run_bass_kernel_spmd(nc, in_maps, core_ids=list(range(N_CORES)))
    outs = [res.results[i]["out"] for i in range(N_CORES)]
    # psum = QS*out + 128.5; both engines' plain f32->u8 casts round to
    # nearest (measured), so q = rne(psum) and reconstruction is q - 128.5.
    full = np.concatenate(outs, axis=0).astype(np.float32)
    full -= 128.5
    full *= 1.0 / QS
    return full.reshape(B_FULL, N_CHAINS, N_JOINTS, N_VERTS, 3)
